# revision 1
# baseline (speedup 1.0000x reference)
"""Multi-head attention kernel for 8 TRN2 NeuronCores.

Problem: B=2, S=2048, D=1024, H=16 heads (HD=64).
  q/k/v = x @ W{q,k,v}.T + b;  out = softmax(q k^T / 8 + mask) v  (heads merged)

Sharding: core c owns batch b = c // 4 and head group g = c % 4 (4 heads,
256 channels).  No collectives: each core computes its [2048, 256] slice of
the output; the host gathers.

Per-core dataflow (all matmul operands fp16, accumulation fp32 in PSUM):
  phase A: Q^T/K^T = We^T x^T via PE (x^T streamed, weights stationary),
           epilogue on ACT (copy + per-partition bias -> fp16 SBUF);
           V = x W + b via PE (ones-row trick for bias), DVE copy into
           V_aug tiles [128, 4*65] (per head: 64 V columns + a ones column).
  phase B, per head h:
    B1: S^T[k,q] blocks via PE (K^T stationary slice, Q^T moving), then
        ACT exp(0.125*S^T + mask[k]) -> E fp16 (softmax numerator;
        no max-subtraction needed: scores are O(1) by construction).
    B2: ctx^T[65, q] += V_aug[kt]^T @ E[kt] accumulated over k tiles in
        PSUM (V stationary so no per-matmul weight reload; the ones column
        of V_aug yields the softmax denominator as row 64).
  ctx^T [4, 65, 2048] f32 is DMA'd out; the host divides by row 64 and
  transposes into the output slice.
"""

import numpy as np

import concourse.mybir as mybir
import concourse.tile as tile
from concourse import bacc
from concourse.bass_utils import run_bass_kernel_spmd

B, S, D, H = 2, 2048, 1024, 16
HD = D // H           # 64
EPC = 256             # e-channels per core (4 heads)
HPC = 4               # heads per core
NDT = D // 128        # 8 d tiles
NKT = S // 128        # 16 k tiles
W = 1024              # exp width (psum tile [128, W], 2 banks)

f32 = mybir.dt.float32
fp16 = mybir.dt.float16

_CACHE = {}


def _build(reps=1, trace_sim=False, loop=False, cache=True):
    key = (reps, loop)
    if cache and key in _CACHE:
        return _CACHE[key]

    nc = bacc.Bacc("TRN2", target_bir_lowering=False, debug=False, num_devices=8)

    # xt is laid out [128, NDT*S] host-side so one DMA moves the whole tensor
    xt_d = nc.dram_tensor("xt", [128, NDT * S], fp16, kind="ExternalInput")
    wq_d = nc.dram_tensor("wq", [128, NDT * EPC], fp16, kind="ExternalInput")
    wk_d = nc.dram_tensor("wk", [128, NDT * EPC], fp16, kind="ExternalInput")
    wv_d = nc.dram_tensor("wv", [128, NDT * EPC], fp16, kind="ExternalInput")
    bq_d = nc.dram_tensor("bq", [128, 2], f32, kind="ExternalInput")
    bk_d = nc.dram_tensor("bk", [128, 2], f32, kind="ExternalInput")
    bv_d = nc.dram_tensor("bv", [1, EPC], fp16, kind="ExternalInput")
    mask_d = nc.dram_tensor("mask", [128, NKT], f32, kind="ExternalInput")
    ctxt_d = nc.dram_tensor("ctxt", [HPC, HD + 1, S], f32, kind="ExternalOutput")

    Exp = mybir.ActivationFunctionType.Exp
    Copy = mybir.ActivationFunctionType.Identity

    with tile.TileContext(nc, trace_sim=trace_sim) as tc:
        with tc.tile_pool(name="sb", bufs=1) as sb, \
             tc.tile_pool(name="wpool", bufs=1) as wpool, \
             tc.tile_pool(name="xpool", bufs=1) as xpool, \
             tc.tile_pool(name="epool", bufs=28) as epool, \
             tc.tile_pool(name="cout", bufs=4) as cout, \
             tc.tile_pool(name="psS", bufs=2, space="PSUM") as psS:

            # ---- loads ----
            mask_sb = sb.tile([128, NKT], f32, tag="mask")
            nc.sync.dma_start(out=mask_sb, in_=mask_d[:, :])
            bq_sb = sb.tile([128, 2], f32, tag="bq")
            nc.sync.dma_start(out=bq_sb, in_=bq_d[:, :])
            bk_sb = sb.tile([128, 2], f32, tag="bk")
            nc.sync.dma_start(out=bk_sb, in_=bk_d[:, :])
            bv_sb = sb.tile([1, EPC], fp16, tag="bv")
            nc.sync.dma_start(out=bv_sb, in_=bv_d[:, :])
            ones_sb = sb.tile([1, 128], fp16, tag="ones")
            nc.vector.memset(ones_sb, 1.0)

            wq_sb, wk_sb, wv_sb = [], [], []
            wv_big = None
            for name, lst, dram in (("wq", wq_sb, wq_d), ("wk", wk_sb, wk_d), ("wv", wv_sb, wv_d)):
                big = wpool.tile([128, NDT * EPC], fp16, tag=name, name=name)
                if name != "wv":
                    nc.sync.dma_start(out=big, in_=dram[:, :])
                else:
                    wv_big = big
                lst.extend(big[:, d * EPC:(d + 1) * EPC] for d in range(NDT))
            xt_big = xpool.tile([128, NDT * S], fp16, tag="xt", name="xt")
            xt_r = xt_big.rearrange("p (d s) -> p d s", d=NDT)
            xtd_r = xt_d.rearrange("p (d s) -> p d s", d=NDT)
            for sc in range(4):
                nc.sync.dma_start(
                    out=xt_r[:, :, sc * 512:(sc + 1) * 512],
                    in_=xtd_r[:, :, sc * 512:(sc + 1) * 512],
                )
            xt_sb = [xt_big[:, d * S:(d + 1) * S] for d in range(NDT)]
            nc.sync.dma_start(out=wv_big, in_=wv_d[:, :])

            qt_sb = [sb.tile([128, S], fp16, tag=f"qt{h}", name=f"qt{h}") for h in range(HPC)]
            kt_sb = [sb.tile([128, S], fp16, tag=f"kt{h}", name=f"kt{h}") for h in range(HPC)]
            vaug = [sb.tile([128, HPC * (HD + 1)], fp16, tag=f"vaug{st}", name=f"vaug{st}")
                    for st in range(NKT)]

            import contextlib

            def emit_body(tc, rep):
              with tc.tile_pool(name=f"psPH{rep}", bufs=4, space="PSUM") as psPH:

                def qkt_etile(et, w_sb, b_sb, dsts, chunks=(0, 1, 2, 3)):
                    """Q^T/K^T e-tile -> per-head tiles [128, S], each head's 64
                    rows duplicated into both partition halves (so S^T matmuls
                    contract over K=128 at full SBUF bandwidth; scores double,
                    absorbed into the exp scale)."""
                    pss = {qc: psPH.tile([128, 512], f32, tag="ph", name="ph")
                           for qc in chunks}
                    for d in range(NDT):
                        for qc in chunks:
                            nc.tensor.matmul(
                                pss[qc],
                                w_sb[d][:, et * 128:(et + 1) * 128],
                                xt_sb[d][:, qc * 512:(qc + 1) * 512],
                                start=(d == 0), stop=(d == NDT - 1),
                            )
                    for qc in chunks:
                        cs = slice(qc * 512, (qc + 1) * 512)
                        for i, h in enumerate((2 * et, 2 * et + 1)):
                            src = pss[qc][i * 64:(i + 1) * 64, :]
                            bias = b_sb[i * 64:(i + 1) * 64, et:et + 1]
                            nc.vector.tensor_scalar_add(dsts[h][0:64, cs], src, bias)
                            nc.vector.tensor_scalar_add(dsts[h][64:128, cs], src, bias)

                def emit_head_b1(h, kts=None, e_tiles=None):
                    if e_tiles is None:
                        e_tiles = []
                    for kt in (kts if kts is not None else range(NKT)):
                        e_t = epool.tile([128, S], fp16, tag="E", name="E")
                        for half in range(S // W):
                            ps = psS.tile([128, W], f32, tag="psS", name="psS")
                            for j in range(W // 512):
                                qc = half * (W // 512) + j
                                nc.tensor.matmul(
                                    ps[:, j * 512:(j + 1) * 512],
                                    kt_sb[h][:, kt * 128:(kt + 1) * 128],
                                    qt_sb[h][:, qc * 512:(qc + 1) * 512],
                                    start=True, stop=True,
                                )
                            # inputs are duplicated across partition halves, so
                            # raw scores are 2x -> scale 1/(2*sqrt(HD))
                            nc.scalar.activation(
                                e_t[:, half * W:(half + 1) * W], ps, Exp,
                                bias=mask_sb[:, kt:kt + 1], scale=0.0625,
                            )
                        e_tiles.append(e_t)
                    return e_tiles

                def emit_head_b2(h, e_tiles, pool):
                    ps = pool.tile([HD + 1, S], f32, tag="psT", name="psT")
                    for kt in range(NKT):
                        for qc in range(S // 512):
                            nc.tensor.matmul(
                                ps[:, qc * 512:(qc + 1) * 512],
                                vaug[kt][:, h * (HD + 1):(h + 1) * (HD + 1)],
                                e_tiles[kt][:, qc * 512:(qc + 1) * 512],
                                start=(kt == 0), stop=(kt == NKT - 1),
                            )
                    for qc in range(S // 512):
                        o = cout.tile([HD + 1, 512], f32, tag="cout", name="cout")
                        nc.vector.tensor_copy(o, ps[:, qc * 512:(qc + 1) * 512])
                        nc.sync.dma_start(out=ctxt_d[h, :, qc * 512:(qc + 1) * 512], in_=o)

                qkt_etile(0, wq_sb, bq_sb, qt_sb, chunks=(0, 1))
                qkt_etile(0, wk_sb, bk_sb, kt_sb, chunks=(0, 1))
                qkt_etile(0, wq_sb, bq_sb, qt_sb, chunks=(2, 3))
                qkt_etile(0, wk_sb, bk_sb, kt_sb, chunks=(2, 3))

                h0_e = emit_head_b1(0)

                # head-1 S^T/exp for the 12 spare E slots (software pipeline:
                # the rest is emitted after B2 h0 frees slots)
                h1_e = emit_head_b1(1, kts=range(0, 12))



                # V tiles (needed by B2; produced kt-major so B2 can chase)
                for st in range(NKT):
                    ps = psPH.tile([128, EPC], f32, tag="ph", name="ph")
                    for d in range(NDT):
                        nc.tensor.matmul(
                            ps, xt_sb[d][:, st * 128:(st + 1) * 128], wv_sb[d],
                            start=(d == 0), stop=False,
                        )
                    nc.tensor.matmul(ps, ones_sb[0:1, :], bv_sb[0:1, :],
                                     start=False, stop=True)
                    vt = vaug[st].rearrange("p (h c) -> p h c", h=HPC)
                    nc.vector.memset(vt[:, :, HD:HD + 1], 1.0)
                    nc.vector.tensor_copy(
                        vt[:, :, 0:HD],
                        ps.rearrange("p (h c) -> p h c", h=HPC),
                    )

                qkt_etile(1, wq_sb, bq_sb, qt_sb)
                qkt_etile(1, wk_sb, bk_sb, kt_sb)

              # psPH closed; psT reuses its banks.  B1(h+1) is emitted in
              # [0:12]/[12:16] segments around B2(h) so exp always has work
              # and E-slot reuse never crosses a not-yet-emitted B2.
              with tc.tile_pool(name=f"psT{rep}", bufs=1, space="PSUM") as psT:
                emit_head_b2(0, h0_e, psT)
                emit_head_b1(1, kts=range(12, NKT), e_tiles=h1_e)
                h2_e = emit_head_b1(2, kts=range(0, 12))
                emit_head_b2(1, h1_e, psT)
                emit_head_b1(2, kts=range(12, NKT), e_tiles=h2_e)
                h3_e = emit_head_b1(3, kts=range(0, 12))
                emit_head_b2(2, h2_e, psT)
                emit_head_b1(3, kts=range(12, NKT), e_tiles=h3_e)
                emit_head_b2(3, h3_e, psT)

            if loop and reps > 1:
                with tc.For_i(0, reps, 1):
                    emit_body(tc, 0)
            else:
                for rep in range(reps):
                    emit_body(tc, rep)

    nc.compile()
    if cache:
        _CACHE[key] = nc
    return nc


def _wlayout(w):
    """[EPC, D] nn.Linear weight slice -> [128, NDT*EPC] (d-tiles along free dim)."""
    wt = w.T.astype(np.float16).reshape(NDT, 128, EPC)
    return np.ascontiguousarray(wt.transpose(1, 0, 2).reshape(128, NDT * EPC))


def _prep_inputs(hidden_states, attn_mask, Wq, bq, Wk, bk, Wv, bv):
    """Build the 8 per-core input maps (host-side sharding)."""
    in_maps = []
    xt_b = {}
    for b in range(B):
        xt = hidden_states[b].T.astype(np.float16).reshape(NDT, 128, S)  # [D, S] tiled
        xt_b[b] = np.ascontiguousarray(xt.transpose(1, 0, 2).reshape(128, NDT * S))
    mask_b = {
        b: np.ascontiguousarray(
            np.asarray(attn_mask[b, 0, 0, :], dtype=np.float32).reshape(NKT, 128).T
        )
        for b in range(B)
    }
    for c in range(8):
        b, g = divmod(c, HPC)
        sl = slice(g * EPC, (g + 1) * EPC)
        in_maps.append({
            "xt": xt_b[b],
            "wq": _wlayout(Wq[sl, :]),
            "wk": _wlayout(Wk[sl, :]),
            "wv": _wlayout(Wv[sl, :]),
            "bq": np.ascontiguousarray(np.asarray(bq[sl], np.float32).reshape(2, 128).T),
            "bk": np.ascontiguousarray(np.asarray(bk[sl], np.float32).reshape(2, 128).T),
            "bv": np.asarray(bv[sl], np.float16).reshape(1, EPC),
            "mask": mask_b[b],
        })
    return in_maps


def kernel(hidden_states, attn_mask, Wq, bq, Wk, bk, Wv, bv):
    hidden_states = np.asarray(hidden_states)
    attn_mask = np.asarray(attn_mask)
    Wq, bq = np.asarray(Wq), np.asarray(bq)
    Wk, bk = np.asarray(Wk), np.asarray(bk)
    Wv, bv = np.asarray(Wv), np.asarray(bv)

    nc = _build()
    in_maps = _prep_inputs(hidden_states, attn_mask, Wq, bq, Wk, bk, Wv, bv)
    res = run_bass_kernel_spmd(nc, in_maps, list(range(8)))

    out = np.empty((B, S, D), np.float32)
    for c in range(8):
        b, g = divmod(c, HPC)
        ctxt = res.results[c]["ctxt"]              # [HPC, 65, S]
        ctx = ctxt[:, :HD, :] / ctxt[:, HD:HD + 1, :]
        # [h, hd, q] -> [q, h*HD+hd]
        out[b, :, g * EPC:(g + 1) * EPC] = ctx.transpose(2, 0, 1).reshape(S, EPC)
    return out



# revision 2
# speedup vs baseline: 1.0126x; 1.0126x over previous
"""Multi-head attention kernel v4 for 8 TRN2 NeuronCores.

v1 structure (per-kt E tiles [128,2048], psT [65,2048], FD=1024 exp with
mask bias) with two changes:
  - B1 processes kt in pairs with chunk-level stationary alternation
    (a,b,a,b): ~238 ns/chunk vs ~291 for v1's same-stationary x4 repeats.
  - fine-grained emission interleave: B2(h-1) kt-steps, V tiles, and the
    et1 projections are emitted BETWEEN B1 pair-rounds so the PE always has
    ready work while ACT drains the exp staging ring (B1 fill 952 ns/round
    vs ACT drain 2294 ns/round leaves ~1.3 us/round of PE slack).
"""

import numpy as np

import concourse.mybir as mybir
import concourse.tile as tile
from concourse import bacc
from concourse.bass_utils import run_bass_kernel_spmd

B, S, D, H = 2, 2048, 1024, 16
HD = D // H           # 64
EPC = 256             # e-channels per core (4 heads)
HPC = 4               # heads per core
NDT = D // 128        # 8 d tiles
NKT = S // 128        # 16 k tiles

f32 = mybir.dt.float32
fp16 = mybir.dt.float16

_CACHE = {}


def _build(reps=1, trace_sim=False, loop=False, cache=True):
    key = (reps, loop)
    if cache and key in _CACHE:
        return _CACHE[key]

    nc = bacc.Bacc("TRN2", target_bir_lowering=False, debug=False, num_devices=8)

    xt_d = nc.dram_tensor("xt", [128, NDT * S], fp16, kind="ExternalInput")
    wq_d = nc.dram_tensor("wq", [128, NDT * EPC], fp16, kind="ExternalInput")
    wk_d = nc.dram_tensor("wk", [128, NDT * EPC], fp16, kind="ExternalInput")
    wv_d = nc.dram_tensor("wv", [128, NDT * EPC], fp16, kind="ExternalInput")
    bq_d = nc.dram_tensor("bq", [128, 2], f32, kind="ExternalInput")
    bk_d = nc.dram_tensor("bk", [128, 2], f32, kind="ExternalInput")
    bv_d = nc.dram_tensor("bv", [1, EPC], fp16, kind="ExternalInput")
    mask_d = nc.dram_tensor("mask", [128, NKT], f32, kind="ExternalInput")
    ctxt_d = nc.dram_tensor("ctxt", [HPC, HD + 1, S], f32, kind="ExternalOutput")

    Exp = mybir.ActivationFunctionType.Exp

    with tile.TileContext(nc, trace_sim=trace_sim) as tc:
        with tc.tile_pool(name="sb", bufs=1) as sb, \
             tc.tile_pool(name="wpool", bufs=1) as wpool, \
             tc.tile_pool(name="xpool", bufs=1) as xpool, \
             tc.tile_pool(name="epool", bufs=28) as epool, \
             tc.tile_pool(name="cout", bufs=4) as cout, \
             tc.tile_pool(name="psS", bufs=2, space="PSUM") as psS:

            # ---- loads: xt chunk 0 + wq/wk first (first proj gates ACT) ----
            xt_big = xpool.tile([128, NDT * S], fp16, tag="xt", name="xt")
            xt_r = xt_big.rearrange("p (d s) -> p d s", d=NDT)
            xtd_r = xt_d.rearrange("p (d s) -> p d s", d=NDT)
            nc.sync.dma_start(out=xt_r[:, :, 0:512], in_=xtd_r[:, :, 0:512])
            wq_big = wpool.tile([128, NDT * EPC], fp16, tag="wq", name="wq")
            nc.sync.dma_start(out=wq_big, in_=wq_d[:, :])
            wk_big = wpool.tile([128, NDT * EPC], fp16, tag="wk", name="wk")
            nc.sync.dma_start(out=wk_big, in_=wk_d[:, :])
            nc.sync.dma_start(out=xt_r[:, :, 512:1024], in_=xtd_r[:, :, 512:1024])
            bq_sb = sb.tile([128, 2], f32, tag="bq")
            nc.sync.dma_start(out=bq_sb, in_=bq_d[:, :])
            bk_sb = sb.tile([128, 2], f32, tag="bk")
            nc.sync.dma_start(out=bk_sb, in_=bk_d[:, :])
            mask_sb = sb.tile([128, NKT], f32, tag="mask")
            nc.sync.dma_start(out=mask_sb, in_=mask_d[:, :])
            for sc in (2, 3):
                nc.sync.dma_start(
                    out=xt_r[:, :, sc * 512:(sc + 1) * 512],
                    in_=xtd_r[:, :, sc * 512:(sc + 1) * 512],
                )
            xt_sb = [xt_big[:, d * S:(d + 1) * S] for d in range(NDT)]
            wv_big = wpool.tile([128, NDT * EPC], fp16, tag="wv", name="wv")
            nc.sync.dma_start(out=wv_big, in_=wv_d[:, :])
            bv_sb = sb.tile([1, EPC], fp16, tag="bv")
            nc.sync.dma_start(out=bv_sb, in_=bv_d[:, :])
            ones_sb = sb.tile([1, 128], fp16, tag="ones")
            nc.vector.memset(ones_sb, 1.0)

            wq_sb = [wq_big[:, d * EPC:(d + 1) * EPC] for d in range(NDT)]
            wk_sb = [wk_big[:, d * EPC:(d + 1) * EPC] for d in range(NDT)]
            wv_sb = [wv_big[:, d * EPC:(d + 1) * EPC] for d in range(NDT)]

            qt_sb = [sb.tile([128, S], fp16, tag=f"qt{h}", name=f"qt{h}") for h in range(HPC)]
            kt_sb = [sb.tile([128, S], fp16, tag=f"kt{h}", name=f"kt{h}") for h in range(HPC)]
            vaug = [sb.tile([128, HPC * (HD + 1)], fp16, tag=f"vaug{st}", name=f"vaug{st}")
                    for st in range(NKT)]

            def emit_body(tc, rep):
              e_tiles = {}      # (h, kt) -> [128, S] fp16 tile
              psT_cur = {}      # h -> psT tile

              def b1_pair(h, p):
                  """kt pair (2p, 2p+1): chunk-level a,b,a,b alternation."""
                  ka, kb = 2 * p, 2 * p + 1
                  eA = e_tiles.setdefault((h, ka),
                                          epool.tile([128, S], fp16, tag="E", name="E"))
                  eB = e_tiles.setdefault((h, kb),
                                          epool.tile([128, S], fp16, tag="E", name="E"))
                  for half in range(2):
                      psA = psS.tile([128, 1024], f32, tag="psS", name="psS")
                      psB = psS.tile([128, 1024], f32, tag="psS", name="psS")
                      for j in range(2):
                          qc = half * 2 + j
                          nc.tensor.matmul(
                              psA[:, j * 512:(j + 1) * 512],
                              kt_sb[h][:, ka * 128:(ka + 1) * 128],
                              qt_sb[h][:, qc * 512:(qc + 1) * 512],
                              start=True, stop=True)
                          nc.tensor.matmul(
                              psB[:, j * 512:(j + 1) * 512],
                              kt_sb[h][:, kb * 128:(kb + 1) * 128],
                              qt_sb[h][:, qc * 512:(qc + 1) * 512],
                              start=True, stop=True)
                      # duplicated q/k halves double the scores -> scale 1/16
                      nc.scalar.activation(eA[:, half * 1024:(half + 1) * 1024], psA,
                                           Exp, bias=mask_sb[:, ka:ka + 1], scale=0.0625)
                      nc.scalar.activation(eB[:, half * 1024:(half + 1) * 1024], psB,
                                           Exp, bias=mask_sb[:, kb:kb + 1], scale=0.0625)

              def b2_step(h, kt, pool):
                  if kt == 0:
                      psT_cur[h] = pool.tile([HD + 1, S], f32, tag="psT", name="psT")
                  ps = psT_cur[h]
                  e_t = e_tiles[(h, kt)]
                  for qc in range(4):
                      nc.tensor.matmul(
                          ps[:, qc * 512:(qc + 1) * 512],
                          vaug[kt][:, h * (HD + 1):(h + 1) * (HD + 1)],
                          e_t[:, qc * 512:(qc + 1) * 512],
                          start=(kt == 0), stop=(kt == NKT - 1))
                  if kt == NKT - 1:
                      for qc in range(4):
                          o = cout.tile([HD + 1, 512], f32, tag="cout", name="cout")
                          nc.vector.tensor_copy(o, ps[:, qc * 512:(qc + 1) * 512])
                          nc.sync.dma_start(out=ctxt_d[h, :, qc * 512:(qc + 1) * 512], in_=o)

              with tc.tile_pool(name=f"psPH{rep}", bufs=4, space="PSUM") as psPH:

                def qkt_etile(et, w_sb, b_sb, dsts, chunks=(0, 1, 2, 3)):
                    pss = {qc: psPH.tile([128, 512], f32, tag="ph", name="ph")
                           for qc in chunks}
                    for d in range(NDT):
                        for qc in chunks:
                            nc.tensor.matmul(
                                pss[qc],
                                w_sb[d][:, et * 128:(et + 1) * 128],
                                xt_sb[d][:, qc * 512:(qc + 1) * 512],
                                start=(d == 0), stop=(d == NDT - 1),
                            )
                    for qc in chunks:
                        cs = slice(qc * 512, (qc + 1) * 512)
                        for i, h in enumerate((2 * et, 2 * et + 1)):
                            src = pss[qc][i * 64:(i + 1) * 64, :]
                            bias = b_sb[i * 64:(i + 1) * 64, et:et + 1]
                            nc.vector.tensor_scalar_add(dsts[h][0:64, cs], src, bias)
                            nc.vector.tensor_scalar_add(dsts[h][64:128, cs], src, bias)

                def emit_v(sts):
                    for st in sts:
                        ps = psPH.tile([128, EPC], f32, tag="ph", name="ph")
                        for d in range(NDT):
                            nc.tensor.matmul(
                                ps, xt_sb[d][:, st * 128:(st + 1) * 128], wv_sb[d],
                                start=(d == 0), stop=False,
                            )
                        nc.tensor.matmul(ps, ones_sb[0:1, :], bv_sb[0:1, :],
                                         start=False, stop=True)
                        vt = vaug[st].rearrange("p (h c) -> p h c", h=HPC)
                        nc.vector.memset(vt[:, :, HD:HD + 1], 1.0)
                        nc.vector.tensor_copy(
                            vt[:, :, 0:HD],
                            ps.rearrange("p (h c) -> p h c", h=HPC),
                        )

                qkt_etile(0, wq_sb, bq_sb, qt_sb, chunks=(0, 1))
                qkt_etile(0, wk_sb, bk_sb, kt_sb, chunks=(0, 1))
                qkt_etile(0, wq_sb, bq_sb, qt_sb, chunks=(2, 3))
                qkt_etile(0, wk_sb, bk_sb, kt_sb, chunks=(2, 3))

                # h0 B1 interleaved with V production
                for p in range(8):
                    b1_pair(0, p)
                    emit_v((2 * p, 2 * p + 1))

                qkt_etile(1, wq_sb, bq_sb, qt_sb, chunks=(0, 1))
                b1_pair(1, 0)
                qkt_etile(1, wk_sb, bk_sb, kt_sb, chunks=(0, 1))
                b1_pair(1, 1)
                qkt_etile(1, wq_sb, bq_sb, qt_sb, chunks=(2, 3))
                b1_pair(1, 2)
                qkt_etile(1, wk_sb, bk_sb, kt_sb, chunks=(2, 3))
                b1_pair(1, 3)

              with tc.tile_pool(name=f"psT{rep}", bufs=1, space="PSUM") as psT:
                # h1 pairs 4-7 carry B2(h0) kt-steps (4 per pair)
                for p in range(4, 8):
                    b1_pair(1, p)
                    for kt in range(4 * (p - 4), 4 * (p - 3)):
                        b2_step(0, kt, psT)
                # h2 pairs carry B2(h1), h3 pairs carry B2(h2)
                for hh, hb in ((2, 1), (3, 2)):
                    for p in range(8):
                        b1_pair(hh, p)
                        b2_step(hb, 2 * p, psT)
                        b2_step(hb, 2 * p + 1, psT)
                # tail: B2(h3)
                for kt in range(NKT):
                    b2_step(3, kt, psT)

            if loop and reps > 1:
                with tc.For_i(0, reps, 1):
                    emit_body(tc, 0)
            else:
                for rep in range(reps):
                    emit_body(tc, rep)

    nc.compile()
    if cache:
        _CACHE[key] = nc
    return nc


def _wlayout(w):
    wt = w.T.astype(np.float16).reshape(NDT, 128, EPC)
    return np.ascontiguousarray(wt.transpose(1, 0, 2).reshape(128, NDT * EPC))


def _prep_inputs(hidden_states, attn_mask, Wq, bq, Wk, bk, Wv, bv):
    in_maps = []
    xt_b = {}
    for b in range(B):
        xt = hidden_states[b].T.astype(np.float16).reshape(NDT, 128, S)
        xt_b[b] = np.ascontiguousarray(xt.transpose(1, 0, 2).reshape(128, NDT * S))
    mask_b = {
        b: np.ascontiguousarray(
            np.asarray(attn_mask[b, 0, 0, :], dtype=np.float32).reshape(NKT, 128).T
        )
        for b in range(B)
    }
    for c in range(8):
        b, g = divmod(c, HPC)
        sl = slice(g * EPC, (g + 1) * EPC)
        in_maps.append({
            "xt": xt_b[b],
            "wq": _wlayout(Wq[sl, :]),
            "wk": _wlayout(Wk[sl, :]),
            "wv": _wlayout(Wv[sl, :]),
            "bq": np.ascontiguousarray(np.asarray(bq[sl], np.float32).reshape(2, 128).T),
            "bk": np.ascontiguousarray(np.asarray(bk[sl], np.float32).reshape(2, 128).T),
            "bv": np.asarray(bv[sl], np.float16).reshape(1, EPC),
            "mask": mask_b[b],
        })
    return in_maps


def kernel(hidden_states, attn_mask, Wq, bq, Wk, bk, Wv, bv):
    hidden_states = np.asarray(hidden_states)
    attn_mask = np.asarray(attn_mask)
    Wq, bq = np.asarray(Wq), np.asarray(bq)
    Wk, bk = np.asarray(Wk), np.asarray(bk)
    Wv, bv = np.asarray(Wv), np.asarray(bv)

    nc = _build()
    in_maps = _prep_inputs(hidden_states, attn_mask, Wq, bq, Wk, bk, Wv, bv)
    res = run_bass_kernel_spmd(nc, in_maps, list(range(8)))

    out = np.empty((B, S, D), np.float32)
    for c in range(8):
        b, g = divmod(c, HPC)
        ctxt = res.results[c]["ctxt"]              # [HPC, 65, S]
        ctx = ctxt[:, :HD, :] / ctxt[:, HD:HD + 1, :]
        out[b, :, g * EPC:(g + 1) * EPC] = ctx.transpose(2, 0, 1).reshape(S, EPC)
    return out


# revision 5
# speedup vs baseline: 1.1131x; 1.0993x over previous
"""Multi-head attention kernel for 8 TRN2 NeuronCores (v7 = v1 + zero-upper-half epilogue, wider cout)."""

import numpy as np

import concourse.mybir as mybir
import concourse.tile as tile
from concourse import bacc
from concourse.bass_utils import run_bass_kernel_spmd

B, S, D, H = 2, 2048, 1024, 16
HD = D // H           # 64
EPC = 256             # e-channels per core (4 heads)
HPC = 4               # heads per core
NDT = D // 128        # 8 d tiles
NKT = S // 128        # 16 k tiles
W = 1024              # exp width (psum tile [128, W], 2 banks)

f32 = mybir.dt.float32
fp16 = mybir.dt.float16

_CACHE = {}


def _build(reps=1, trace_sim=False, loop=False, cache=True):
    key = (reps, loop)
    if cache and key in _CACHE:
        return _CACHE[key]

    nc = bacc.Bacc("TRN2", target_bir_lowering=False, debug=False, num_devices=8)

    xt_d = nc.dram_tensor("xt", [128, NDT * S], fp16, kind="ExternalInput")
    wq_d = nc.dram_tensor("wq", [128, NDT * EPC], fp16, kind="ExternalInput")
    wk_d = nc.dram_tensor("wk", [128, NDT * EPC], fp16, kind="ExternalInput")
    wv_d = nc.dram_tensor("wv", [128, NDT * EPC], fp16, kind="ExternalInput")
    bq_d = nc.dram_tensor("bq", [128, 2], f32, kind="ExternalInput")
    bk_d = nc.dram_tensor("bk", [128, 2], f32, kind="ExternalInput")
    bv_d = nc.dram_tensor("bv", [1, EPC], fp16, kind="ExternalInput")
    mask_d = nc.dram_tensor("mask", [128, NKT], f32, kind="ExternalInput")
    ctxt_d = nc.dram_tensor("ctxt", [HPC, HD + 1, S], f32, kind="ExternalOutput")

    Exp = mybir.ActivationFunctionType.Exp

    with tile.TileContext(nc, trace_sim=trace_sim) as tc:
        with tc.tile_pool(name="sb", bufs=1) as sb, \
             tc.tile_pool(name="wpool", bufs=1) as wpool, \
             tc.tile_pool(name="xpool", bufs=1) as xpool, \
             tc.tile_pool(name="epool", bufs=28) as epool, \
             tc.tile_pool(name="cout", bufs=2) as cout, \
             tc.tile_pool(name="psS", bufs=2, space="PSUM") as psS:

            mask_sb = sb.tile([128, NKT], f32, tag="mask")
            nc.sync.dma_start(out=mask_sb, in_=mask_d[:, :])
            bq_sb = sb.tile([128, 2], f32, tag="bq")
            nc.sync.dma_start(out=bq_sb, in_=bq_d[:, :])
            bk_sb = sb.tile([128, 2], f32, tag="bk")
            nc.sync.dma_start(out=bk_sb, in_=bk_d[:, :])
            bv_sb = sb.tile([1, EPC], fp16, tag="bv")
            nc.sync.dma_start(out=bv_sb, in_=bv_d[:, :])
            ones_sb = sb.tile([1, 128], fp16, tag="ones")
            nc.vector.memset(ones_sb, 1.0)

            wq_sb, wk_sb, wv_sb = [], [], []
            wv_big = None
            for name, lst, dram in (("wq", wq_sb, wq_d), ("wk", wk_sb, wk_d), ("wv", wv_sb, wv_d)):
                big = wpool.tile([128, NDT * EPC], fp16, tag=name, name=name)
                if name != "wv":
                    nc.sync.dma_start(out=big, in_=dram[:, :])
                else:
                    wv_big = big
                lst.extend(big[:, d * EPC:(d + 1) * EPC] for d in range(NDT))
            xt_big = xpool.tile([128, NDT * S], fp16, tag="xt", name="xt")
            xt_r = xt_big.rearrange("p (d s) -> p d s", d=NDT)
            xtd_r = xt_d.rearrange("p (d s) -> p d s", d=NDT)
            for sc in range(4):
                nc.sync.dma_start(
                    out=xt_r[:, :, sc * 512:(sc + 1) * 512],
                    in_=xtd_r[:, :, sc * 512:(sc + 1) * 512],
                )
            xt_sb = [xt_big[:, d * S:(d + 1) * S] for d in range(NDT)]
            nc.sync.dma_start(out=wv_big, in_=wv_d[:, :])

            qt_sb = [sb.tile([128, S], fp16, tag=f"qt{h}", name=f"qt{h}") for h in range(HPC)]
            kt_sb = [sb.tile([128, S], fp16, tag=f"kt{h}", name=f"kt{h}") for h in range(HPC)]
            # upper halves stay zero -> contribute 0 to the 128-contraction, so
            # the bias epilogue writes only the lower half and scores are 1x
            for h in range(HPC):
                nc.vector.memset(qt_sb[h][64:128, :], 0.0)
                nc.vector.memset(kt_sb[h][64:128, :], 0.0)
            vaug = [sb.tile([128, HPC * (HD + 1)], fp16, tag=f"vaug{st}", name=f"vaug{st}")
                    for st in range(NKT)]

            def emit_body(tc, rep):
              with tc.tile_pool(name=f"psPH{rep}", bufs=4, space="PSUM") as psPH:

                def qkt_etile(et, w_sb, b_sb, dsts, chunks=(0, 1, 2, 3)):
                    pss = {qc: psPH.tile([128, 512], f32, tag="ph", name="ph")
                           for qc in chunks}
                    for d in range(NDT):
                        for qc in chunks:
                            nc.tensor.matmul(
                                pss[qc],
                                w_sb[d][:, et * 128:(et + 1) * 128],
                                xt_sb[d][:, qc * 512:(qc + 1) * 512],
                                start=(d == 0), stop=(d == NDT - 1),
                            )
                    for qc in chunks:
                        cs = slice(qc * 512, (qc + 1) * 512)
                        for i, h in enumerate((2 * et, 2 * et + 1)):
                            src = pss[qc][i * 64:(i + 1) * 64, :]
                            bias = b_sb[i * 64:(i + 1) * 64, et:et + 1]
                            nc.vector.tensor_scalar_add(dsts[h][0:64, cs], src, bias)

                def emit_head_b1(h, kts=None, e_tiles=None):
                    if e_tiles is None:
                        e_tiles = []
                    for kt in (kts if kts is not None else range(NKT)):
                        e_t = epool.tile([128, S], fp16, tag="E", name="E")
                        for half in range(S // W):
                            ps = psS.tile([128, W], f32, tag="psS", name="psS")
                            for j in range(W // 512):
                                qc = half * (W // 512) + j
                                nc.tensor.matmul(
                                    ps[:, j * 512:(j + 1) * 512],
                                    kt_sb[h][:, kt * 128:(kt + 1) * 128],
                                    qt_sb[h][:, qc * 512:(qc + 1) * 512],
                                    start=True, stop=True,
                                )
                            nc.scalar.activation(
                                e_t[:, half * W:(half + 1) * W], ps, Exp,
                                bias=mask_sb[:, kt:kt + 1], scale=0.125,
                            )
                        e_tiles.append(e_t)
                    return e_tiles

                def emit_head_b2(h, e_tiles, pool):
                    ps = pool.tile([HD + 1, S], f32, tag="psT", name="psT")
                    for kt in range(NKT):
                        for qc in range(S // 512):
                            nc.tensor.matmul(
                                ps[:, qc * 512:(qc + 1) * 512],
                                vaug[kt][:, h * (HD + 1):(h + 1) * (HD + 1)],
                                e_tiles[kt][:, qc * 512:(qc + 1) * 512],
                                start=(kt == 0), stop=(kt == NKT - 1),
                            )
                    for qh in range(2):
                        o = cout.tile([HD + 1, 1024], f32, tag="cout", name="cout")
                        nc.vector.tensor_copy(o, ps[:, qh * 1024:(qh + 1) * 1024])
                        nc.sync.dma_start(out=ctxt_d[h, :, qh * 1024:(qh + 1) * 1024], in_=o)

                qkt_etile(0, wq_sb, bq_sb, qt_sb, chunks=(0, 1))
                qkt_etile(0, wk_sb, bk_sb, kt_sb, chunks=(0, 1))
                qkt_etile(0, wq_sb, bq_sb, qt_sb, chunks=(2, 3))
                qkt_etile(0, wk_sb, bk_sb, kt_sb, chunks=(2, 3))

                h0_e = emit_head_b1(0)
                h1_e = emit_head_b1(1, kts=range(0, 12))

                for st in range(NKT):
                    ps = psPH.tile([128, EPC], f32, tag="ph", name="ph")
                    for d in range(NDT):
                        nc.tensor.matmul(
                            ps, xt_sb[d][:, st * 128:(st + 1) * 128], wv_sb[d],
                            start=(d == 0), stop=False,
                        )
                    nc.tensor.matmul(ps, ones_sb[0:1, :], bv_sb[0:1, :],
                                     start=False, stop=True)
                    vt = vaug[st].rearrange("p (h c) -> p h c", h=HPC)
                    nc.vector.memset(vt[:, :, HD:HD + 1], 1.0)
                    nc.vector.tensor_copy(
                        vt[:, :, 0:HD],
                        ps.rearrange("p (h c) -> p h c", h=HPC),
                    )

                qkt_etile(1, wq_sb, bq_sb, qt_sb)
                qkt_etile(1, wk_sb, bk_sb, kt_sb)

              with tc.tile_pool(name=f"psT{rep}", bufs=1, space="PSUM") as psT:
                emit_head_b2(0, h0_e, psT)
                emit_head_b1(1, kts=range(12, NKT), e_tiles=h1_e)
                h2_e = emit_head_b1(2, kts=range(0, 12))
                emit_head_b2(1, h1_e, psT)
                emit_head_b1(2, kts=range(12, NKT), e_tiles=h2_e)
                h3_e = emit_head_b1(3, kts=range(0, 12))
                emit_head_b2(2, h2_e, psT)
                emit_head_b1(3, kts=range(12, NKT), e_tiles=h3_e)
                emit_head_b2(3, h3_e, psT)

            if loop and reps > 1:
                with tc.For_i(0, reps, 1):
                    emit_body(tc, 0)
            else:
                for rep in range(reps):
                    emit_body(tc, rep)

    nc.compile()
    if cache:
        _CACHE[key] = nc
    return nc



def _wlayout(w):
    """[EPC, D] nn.Linear weight slice -> [128, NDT*EPC] (d-tiles along free dim)."""
    wt = w.T.astype(np.float16).reshape(NDT, 128, EPC)
    return np.ascontiguousarray(wt.transpose(1, 0, 2).reshape(128, NDT * EPC))


def _prep_inputs(hidden_states, attn_mask, Wq, bq, Wk, bk, Wv, bv):
    """Build the 8 per-core input maps (host-side sharding)."""
    in_maps = []
    xt_b = {}
    for b in range(B):
        xt = hidden_states[b].T.astype(np.float16).reshape(NDT, 128, S)
        xt_b[b] = np.ascontiguousarray(xt.transpose(1, 0, 2).reshape(128, NDT * S))
    mask_b = {
        b: np.ascontiguousarray(
            np.asarray(attn_mask[b, 0, 0, :], dtype=np.float32).reshape(NKT, 128).T
        )
        for b in range(B)
    }
    for c in range(8):
        b, g = divmod(c, HPC)
        sl = slice(g * EPC, (g + 1) * EPC)
        in_maps.append({
            "xt": xt_b[b],
            "wq": _wlayout(Wq[sl, :]),
            "wk": _wlayout(Wk[sl, :]),
            "wv": _wlayout(Wv[sl, :]),
            "bq": np.ascontiguousarray(np.asarray(bq[sl], np.float32).reshape(2, 128).T),
            "bk": np.ascontiguousarray(np.asarray(bk[sl], np.float32).reshape(2, 128).T),
            "bv": np.asarray(bv[sl], np.float16).reshape(1, EPC),
            "mask": mask_b[b],
        })
    return in_maps


def kernel(hidden_states, attn_mask, Wq, bq, Wk, bk, Wv, bv):
    hidden_states = np.asarray(hidden_states)
    attn_mask = np.asarray(attn_mask)
    Wq, bq = np.asarray(Wq), np.asarray(bq)
    Wk, bk = np.asarray(Wk), np.asarray(bk)
    Wv, bv = np.asarray(Wv), np.asarray(bv)

    nc = _build()
    in_maps = _prep_inputs(hidden_states, attn_mask, Wq, bq, Wk, bk, Wv, bv)
    res = run_bass_kernel_spmd(nc, in_maps, list(range(8)))

    out = np.empty((B, S, D), np.float32)
    for c in range(8):
        b, g = divmod(c, HPC)
        ctxt = res.results[c]["ctxt"]              # [HPC, 65, S]
        ctx = ctxt[:, :HD, :] / ctxt[:, HD:HD + 1, :]
        out[b, :, g * EPC:(g + 1) * EPC] = ctx.transpose(2, 0, 1).reshape(S, EPC)
    return out
